# revision 13
# baseline (speedup 1.0000x reference)
"""DeepseekV4-style sparse attention on 8 Trainium2 cores (Bass/Tile), v2.

Sharding: data-parallel over batch (2) x tensor-parallel over heads (16 -> 4
groups of 4).  Core c handles batch c//4 and heads [4*(c%4), 4*(c%4)+4).
Per-core partial outputs (attn_heads @ wo_rows) are summed on the host.

v2 structure (vs v1): fully software-pipelined per 512-token chunk so the PE
engine never idles behind the pooling chain:
    for j: emit B_{j-1} (pooling, ACT/DVE) ; A_j (projections, PE) ;
           C_{j-1} (attention + out-proj, PE)
  - softmax denominator is computed as ones[128,128]^T @ exp(S^T), giving the
    denominator replicated across all 128 partitions ("rbden") -- no separate
    broadcast matmul / copy; sink is added via DVE tensor_scalar, then
    reciprocal, then one DVE multiply normalizes PV.
  - kv/gate slabs for pooling live in 2-chunk rings with a RATIO-column halo
    (lo halves only -- the hi halves pool the current window).
  - initial DMAs are split per-k-group and ordered so the first matmul can
    start after ~1/4 of wq + hslab0 arrived.
"""

import numpy as np
import ml_dtypes

import concourse.bass as bass
import concourse.mybir as mybir
import concourse.tile as tile
from concourse.bass import ts
from concourse.masks import make_identity

F32 = mybir.dt.float32
BF16 = mybir.dt.bfloat16
AF = mybir.ActivationFunctionType

B, S, HID, NH, HD, RD, RATIO = 2, 4096, 2048, 16, 128, 64, 4
THETA = 10000.0
NW = S // RATIO
N_CORES = 8
HPC = 4
CW = HPC * HD
TCH = 512
NCH = S // TCH
WCH = 128
KCH = HID // 128
SCALE = HD ** -0.5

_PAIR_SWAP = [i ^ 1 for i in range(32)]


def _build_nc(n_reps: int = 1, split_waits: bool = True):
    nc = bass.Bass(num_devices=N_CORES)
    dp = nc.declare_dram_parameter
    ht = dp("ht", [HID, S], BF16, isOutput=False)
    htkv = dp("htkv", [HID, 2, RATIO + TCH], BF16, isOutput=False)
    gfix = dp("gfix", [128, RATIO], F32, isOutput=False)
    wq = dp("wq", [HID, CW], BF16, isOutput=False)
    wkv = dp("wkv", [HID, 2 * HD], BF16, isOutput=False)
    wg = dp("wg", [HID, 2 * HD], BF16, isOutput=False)
    wo = dp("wo", [CW, HID], BF16, isOutput=False)
    eape = dp("eape", [HD, 2 * RATIO], F32, isOutput=False)
    esinkb = dp("esinkb", [128, HPC], F32, isOutput=False)
    cosq = dp("cosq", [RD, S], BF16, isOutput=False)
    sinq = dp("sinq", [RD, S], BF16, isOutput=False)
    cosk = dp("cosk", [RD, NW], BF16, isOutput=False)
    sink = dp("sink", [RD, NW], BF16, isOutput=False)
    bandm = dp("bandm", [WCH, TCH], BF16, isOutput=False)
    out = dp("out", [S, HID], BF16, isOutput=True)
    args = (ht, htkv, wq, wkv, wg, wo, eape, esinkb, gfix, cosq, sinq, cosk,
            sink, bandm, out)

    with tile.TileContext(nc) as tc:
        for _ in range(n_reps):   # python-unrolled: For_i + collectives
            _body(nc, tc, *args)  # fails walrus codegen
    if split_waits:
        _split_multi_waits(nc)
    return nc


def _body(nc, tc, ht, htkv, wq, wkv, wg, wo, eape, esinkb, gfix,
          cosq, sinq, cosk, sink, bandm, out):
    HLO = RATIO + TCH               # own-chunk slab width (4-col halo)
    GRP = [[0, 1, 2, 3], [4, 5, 6, 7]]
    with (
        tc.tile_pool(name="persist", bufs=1) as pp,
        tc.tile_pool(name="wts", bufs=1) as wts,
        tc.tile_pool(name="hslab", bufs=2) as hs,
        tc.tile_pool(name="ev", bufs=3) as ev,
        tc.tile_pool(name="bwork", bufs=1) as bw,
        tc.tile_pool(name="att", bufs=2) as att,
        tc.tile_pool(name="dram", bufs=2, space="DRAM") as dram,
        tc.tile_pool(name="psAO", bufs=3, space="PSUM") as psAO,
        tc.tile_pool(name="psS", bufs=2, space="PSUM") as psS,
        tc.tile_pool(name="psAcc", bufs=2, space="PSUM") as psAcc,
        tc.tile_pool(name="psRB", bufs=1, space="PSUM") as psRB,
    ):
        # ---- persistent SBUF state ----
        qT = [pp.tile([128, 4, TCH], BF16, tag=f"qT{m}", name=f"qT{m}")
              for m in range(HPC)]
        # own-chunk kv/gate slabs (2 owned chunks, halo cols 0:RATIO)
        okvlo = pp.tile([128, 2, HLO], BF16, tag="okvlo", name="okvlo")
        oglo = pp.tile([128, 2, HLO], BF16, tag="oglo", name="oglo")
        okvhi = pp.tile([128, 2, HLO], BF16, tag="okvhi", name="okvhi")
        oghi = pp.tile([128, 2, HLO], BF16, tag="oghi", name="oghi")
        cosq_s = pp.tile([128, S], BF16, tag="cosq", name="cosq")
        sinq_s = pp.tile([128, S], BF16, tag="sinq", name="sinq")
        cosk_s = pp.tile([128, 2 * WCH], BF16, tag="cosk", name="cosk")
        sink_s = pp.tile([128, 2 * WCH], BF16, tag="sink", name="sink")
        eape_s = pp.tile([HD, 2 * RATIO], F32, tag="eape", name="eape")
        esink_s = pp.tile([128, HPC], F32, tag="esink", name="esink")
        gfix_s = pp.tile([128, RATIO], F32, tag="gfix", name="gfix")
        bandm_s = pp.tile([WCH, TCH], BF16, tag="bandm", name="bandm")
        wo_s = pp.tile([HD, HPC, HID], BF16, tag="wo", name="wo")
        ones_w = pp.tile([WCH, WCH], BF16, tag="ones_w", name="ones_w")
        kT = pp.tile([HD, NW], BF16, tag="kT", name="kT")
        v_s = pp.tile([WCH, NW // WCH, HD], BF16, tag="v", name="v")
        ident = pp.tile([128, 128], F32, tag="ident", name="ident")

        wq_s = wts.tile([128, KCH, CW], BF16, tag="wq", name="wq")
        wkv_s = wts.tile([128, KCH, 2 * HD], BF16, tag="wkv", name="wkv")
        wg_s = wts.tile([128, KCH, 2 * HD], BF16, tag="wg", name="wg")
        htkv_s = [wts.tile([128, KCH, HLO], BF16, tag=f"htkv{oc}",
                           name=f"htkv{oc}") for oc in range(2)]

        # ---- preamble: ordered DMAs (chunk-0-critical first) ----
        htr = ht.rearrange("(k p) t -> p k t", p=128)
        htkvr = htkv.rearrange("(k p) o w -> p k o w", p=128)
        wqr = wq.rearrange("(k p) c -> p k c", p=128)
        hsl0 = hs.tile([128, KCH, TCH], BF16, tag="hslab", name="hslab0")
        for g in range(4):
            ksl = slice(4 * g, 4 * g + 4)
            nc.sync.dma_start(wq_s[:, ksl, :], wqr[:, ksl, :])
            nc.sync.dma_start(hsl0[:, ksl, :], htr[:, ksl, ts(0, TCH)])
        nc.sync.dma_start(wkv_s[:], wkv.rearrange("(k p) c -> p k c", p=128))
        nc.sync.dma_start(wg_s[:], wg.rearrange("(k p) c -> p k c", p=128))
        for g in range(4):
            ksl = slice(4 * g, 4 * g + 4)
            nc.sync.dma_start(htkv_s[0][:, ksl, :], htkvr[:, ksl, 0, :])
        nc.sync.dma_start(cosq_s[RD:128, ts(0, TCH)], cosq[:, ts(0, TCH)])
        nc.sync.dma_start(sinq_s[RD:128, ts(0, TCH)], sinq[:, ts(0, TCH)])
        nc.sync.dma_start(gfix_s[:], gfix[:])
        nc.sync.dma_start(eape_s[:], eape[:])
        nc.sync.dma_start(esink_s[:], esinkb[:])
        nc.sync.dma_start(cosk_s[RD:128, :], cosk[:, 0:2 * WCH])
        nc.sync.dma_start(sink_s[RD:128, :], sink[:, 0:2 * WCH])
        nc.vector.memset(ones_w[:], 1.0)
        make_identity(nc, ident[:])

        hsl_tiles = {0: hsl0}
        pk_outs = {}

        def stage_aq(j):
            """q projection (+RoPE) for chunk j; prefetch next hslab."""
            tsl = ts(j, TCH)
            hsl = hsl_tiles.pop(j)
            if j + 1 < NCH:
                nxt = hs.tile([128, KCH, TCH], BF16, tag="hslab",
                              name=f"hslab{j + 1}")
                for g in range(4):
                    ksl = slice(4 * g, 4 * g + 4)
                    nc.sync.dma_start(nxt[:, ksl, :],
                                      htr[:, ksl, ts(j + 1, TCH)])
                hsl_tiles[j + 1] = nxt
                nc.sync.dma_start(cosq_s[RD:128, ts(j + 1, TCH)],
                                  cosq[:, ts(j + 1, TCH)])
                nc.sync.dma_start(sinq_s[RD:128, ts(j + 1, TCH)],
                                  sinq[:, ts(j + 1, TCH)])
            if j == 0:      # heavy non-critical loads after hslab1 prefetch
                for g in range(4):
                    ksl = slice(4 * g, 4 * g + 4)
                    nc.sync.dma_start(htkv_s[1][:, ksl, :],
                                      htkvr[:, ksl, 1, :])
                nc.sync.dma_start(bandm_s[:], bandm[:])
                nc.sync.dma_start(wo_s[:],
                                  wo.rearrange("(h p) e -> p h e", p=HD))
            sl = j % 4
            for m in range(HPC):
                ps = psAO.tile([128, TCH], F32, tag="a", name="aq")
                for k in range(KCH):
                    nc.tensor.matmul(ps[:], wq_s[:, k, ts(m, 128)],
                                     hsl[:, k, :], start=(k == 0),
                                     stop=(k == KCH - 1))
                nc.scalar.copy(qT[m][0:RD, sl, :], ps[0:RD, :])
                rb = ev.tile([128, TCH], BF16, tag="ropebuf", name="ropebuf", bufs=2)
                nc.vector.tensor_copy(rb[RD:128, :], ps[RD:128, :])
                sw = ev.tile([128, TCH], BF16, tag="ropeswap", name="ropeswap", bufs=2)
                nc.vector.stream_shuffle(sw[RD:128, :], rb[RD:128, :],
                                         _PAIR_SWAP)
                t1 = ev.tile([128, TCH], BF16, tag="ropet1", name="ropet1", bufs=2)
                nc.vector.tensor_mul(t1[RD:128, :], rb[RD:128, :],
                                     cosq_s[RD:128, tsl])
                t2 = ev.tile([128, TCH], BF16, tag="ropet2", name="ropet2", bufs=2)
                nc.vector.tensor_mul(t2[RD:128, :], sw[RD:128, :],
                                     sinq_s[RD:128, tsl])
                nc.vector.tensor_add(qT[m][RD:128, sl, :], t1[RD:128, :],
                                     t2[RD:128, :])

        def stage_own(oc):
            """kv/gate projection + pooling + rope/transpose + pack for the
            core's own chunk `oc` (0 or 1); ends with the pack DMA to DRAM."""
            for dst, col, wsrc in (
                (okvlo, slice(0, 128), wkv_s),
                (okvhi, slice(128, 256), wkv_s),
                (oglo, slice(0, 128), wg_s),
                (oghi, slice(128, 256), wg_s),
            ):
                ps = psAO.tile([128, TCH], F32, tag="a", name="okv")
                for k in range(KCH):
                    nc.tensor.matmul(ps[:], wsrc[:, k, col],
                                     htkv_s[oc][:, k, 0:TCH],
                                     start=(k == 0), stop=(k == KCH - 1))
                psh = psAO.tile([128, TCH], F32, tag="a", name="okvh")
                for k in range(KCH):
                    nc.tensor.matmul(psh[:, 0:RATIO], wsrc[:, k, col],
                                     htkv_s[oc][:, k, TCH:HLO],
                                     start=(k == 0), stop=(k == KCH - 1))
                nc.scalar.copy(dst[:, oc, 0:TCH], ps[:])
                nc.scalar.copy(dst[:, oc, TCH:HLO], psh[:, 0:RATIO])
            if oc == 0:   # chunk-0 halo gate -> -inf on the owning cores
                nc.vector.tensor_add(oglo[:, 0, 0:RATIO],
                                     oglo[:, 0, 0:RATIO], gfix_s[:])


            # pooling (windows of the owned chunk)
            numer = bw.tile([HD, WCH], F32, tag="numer", name="numer")
            denom = bw.tile([HD, WCH], F32, tag="denom", name="denom")
            for half, (g_src, kv_src, acol, off) in enumerate((
                (oglo[:, oc, :], okvlo[:, oc, :], slice(0, RATIO), 0),
                (oghi[:, oc, :], okvhi[:, oc, :], slice(RATIO, 2 * RATIO),
                 RATIO),
            )):
                e = bw.tile([HD, HLO], F32, tag="poole", name=f"poole{half}")
                nc.scalar.activation(e[:], g_src[:], AF.Exp)
                nc.vector.tensor_mul(
                    e[:].rearrange("d (w r) -> d w r", r=RATIO),
                    e[:].rearrange("d (w r) -> d w r", r=RATIO),
                    eape_s[:, None, acol].to_broadcast(
                        [HD, HLO // RATIO, RATIO]))
                ea = bw.tile([HD, HLO], F32, tag="poolea",
                             name=f"poolea{half}")
                nc.vector.tensor_mul(ea[:], e[:], kv_src[:])
                for acc, src in ((denom, e), (numer, ea)):
                    s3 = src[:, off:off + TCH].rearrange(
                        "d (w r) -> d w r", r=RATIO)
                    nm = f"poolred{half}{1 if acc is numer else 0}"
                    ra = bw.tile([HD, WCH], F32, tag="poolra", name=nm + "a")
                    nc.vector.tensor_add(ra[:], s3[:, :, 0], s3[:, :, 1])
                    rc = bw.tile([HD, WCH], F32, tag="poolrc", name=nm + "c")
                    nc.vector.tensor_add(rc[:], s3[:, :, 2], s3[:, :, 3])
                    if half == 0:
                        nc.vector.tensor_add(acc[:], ra[:], rc[:])
                    else:
                        nc.vector.tensor_add(ra[:], ra[:], rc[:])
                        nc.vector.tensor_add(acc[:], acc[:], ra[:])
            rec = bw.tile([HD, WCH], F32, tag="poolrec", name="poolrec")
            nc.vector.reciprocal(rec[:], denom[:])
            pooledc = bw.tile([HD, WCH], F32, tag="pooledc", name="pooledc")
            nc.vector.tensor_mul(pooledc[:], numer[:], rec[:])

            pack = bw.tile([128, 2 * WCH], BF16, tag="pack", name="pack",
                           bufs=2)
            # cols 0:WCH = roped kT chunk; WCH:2*WCH = V chunk (transposed).
            # rope needs per-chunk position tables: owned chunk index is
            # data-dependent, so host sends cosk/sink laid out per OWN chunk.
            nc.scalar.copy(pack[0:RD, 0:WCH], pooledc[0:RD, :])
            krb = bw.tile([128, WCH], BF16, tag="krope", name="krope")
            nc.scalar.copy(krb[RD:128, :], pooledc[RD:128, :])
            ksw = bw.tile([128, WCH], BF16, tag="kswap", name="kswap")
            nc.vector.stream_shuffle(ksw[RD:128, :], krb[RD:128, :],
                                     _PAIR_SWAP)
            kt1 = bw.tile([128, WCH], BF16, tag="kt1", name="kt1")
            nc.vector.tensor_mul(kt1[RD:128, :], krb[RD:128, :],
                                 cosk_s[RD:128, ts(oc, WCH)])
            kt2 = bw.tile([128, WCH], BF16, tag="kt2", name="kt2")
            nc.vector.tensor_mul(kt2[RD:128, :], ksw[RD:128, :],
                                 sink_s[RD:128, ts(oc, WCH)])
            nc.vector.tensor_add(pack[RD:128, 0:WCH], kt1[RD:128, :],
                                 kt2[RD:128, :])
            pk_in = dram.tile([128, 2 * WCH], BF16, tag="pk_in",
                              name=f"pk_in{oc}")
            nc.gpsimd.dma_start(pk_in[:, 0:WCH], pack[:, 0:WCH])
            return pooledc, pack, pk_in

        def stage_own_fin(oc, pooledc, pack, pk_in):
            # deferred PE transpose (runs after the next q-projection)
            tp = psAO.tile([128, TCH], F32, tag="a", name="vtrans")
            nc.tensor.transpose(tp[:, 0:WCH], pooledc[:], ident[:])
            nc.scalar.copy(pack[:, WCH:2 * WCH], tp[:, 0:WCH])
            nc.gpsimd.dma_start(pk_in[:, WCH:2 * WCH], pack[:, WCH:2 * WCH])
            return pk_in

        def gather(oc, pk_in):
            pk_out = dram.tile([4 * 128, 2 * WCH], BF16, tag="pk_out",
                               name=f"pk_out{oc}")
            nc.gpsimd.collective_compute(
                "AllGather", mybir.AluOpType.bypass, replica_groups=GRP,
                ins=[pk_in[:].opt()], outs=[pk_out[:].opt()])
            pk_outs[oc] = pk_out

        def unpack(oc):
            pk_out = pk_outs.pop(oc)
            for r in range(4):
                ch = 4 * oc + r
                nc.sync.dma_start(kT[:, ts(ch, WCH)],
                                  pk_out[128 * r:128 * (r + 1), 0:WCH])
                nc.sync.dma_start(v_s[:, ch, :],
                                  pk_out[128 * r:128 * (r + 1), WCH:2 * WCH])

        def stage_c(jc):
            """Attention + output projection for query chunk jc."""
            tsl = ts(jc, TCH)
            a_sb = []
            for h in range(HPC):
                qsl = qT[h][:, jc % 4, :]
                rbden = psRB.tile([128, TCH], F32, tag="rb", name="rbden")
                acc_ps = psAcc.tile([128, TCH], F32, tag="acc", name="acc")
                s_tiles = {}

                def _score(wc, s_tiles=s_tiles, qsl=qsl):
                    sp = psS.tile([WCH, TCH], F32, tag="s", name="s")
                    nc.tensor.matmul(sp[:], kT[:, ts(wc, WCH)], qsl,
                                     start=True, stop=True)
                    s_tiles[wc] = sp

                _score(0)
                for wc in range(jc + 1):
                    if wc < jc:
                        _score(wc + 1)
                    s_ps = s_tiles.pop(wc)
                    pt = ev.tile([WCH, TCH], BF16, tag="pt", name="pt")
                    nc.scalar.activation(pt[:], s_ps[:], AF.Exp, scale=SCALE)
                    if wc == jc:
                        ptm = ev.tile([WCH, TCH], BF16, tag="ptm", name="ptm")
                        nc.vector.tensor_mul(ptm[:], pt[:], bandm_s[:])
                        pt = ptm
                    nc.tensor.matmul(rbden[:], ones_w[:], pt[:],
                                     start=(wc == 0), stop=(wc == jc))
                    nc.tensor.matmul(acc_ps[:], v_s[:, wc, :], pt[:],
                                     start=(wc == 0), stop=(wc == jc))
                dsink = att.tile([128, TCH], F32, tag="dsink", name="dsink")
                nc.scalar.activation(dsink[:], rbden[:], AF.Identity,
                                     bias=esink_s[:, h:h + 1])
                rec_sb = att.tile([128, TCH], F32, tag="rec_sb", name="rec_sb")
                nc.vector.reciprocal(rec_sb[:], dsink[:])
                a = att.tile([128, TCH], BF16, tag=f"attnT{h}",
                             name=f"attnT{h}")
                nc.vector.tensor_mul(a[:], acc_ps[:], rec_sb[:])
                a_sb.append(a)

            for tt in range(TCH // 128):
                for e in range(HID // TCH):
                    o_ps = psAO.tile([128, TCH], F32, tag="a", name="o")
                    for h in range(HPC):
                        nc.tensor.matmul(o_ps[:], a_sb[h][:, ts(tt, 128)],
                                         wo_s[:, h, ts(e, TCH)],
                                         start=(h == 0), stop=(h == HPC - 1))
                    o_sb = att.tile([128, TCH], BF16, tag="o_sb", name="o_sb",
                                    bufs=2)
                    if e % 2 == 0:
                        nc.scalar.copy(o_sb[:], o_ps[:])
                    else:
                        nc.vector.tensor_copy(o_sb[:], o_ps[:])
                    nc.sync.dma_start(
                        out[jc * TCH + tt * 128:jc * TCH + (tt + 1) * 128,
                            ts(e, TCH)], o_sb[:])

        pk0 = pk1 = None
        for j in range(NCH):
            stage_aq(j)
            if j == 0:
                own0 = stage_own(0)
            elif j == 1:
                pk0 = stage_own_fin(0, *own0)
                own1 = stage_own(1)
                gather(0, pk0)
                unpack(0)
            elif j == 2:
                pk1 = stage_own_fin(1, *own1)
                gather(1, pk1)
                unpack(1)
            if j >= 3:
                stage_c(j - 3)
        for jc in range(NCH - 3, NCH):
            stage_c(jc)
_WS_CTR = [0]


def _split_multi_waits(nc):
    """This walrus build accepts at most ONE sync wait per instruction; hoist
    extras onto same-engine NOPs placed immediately before."""
    f = nc.m.functions[0]
    for blk in f.blocks:
        insts = blk.instructions
        if not any(i.sync_info is not None and len(i.sync_info.on_wait) > 1
                   for i in insts):
            continue
        new_list = []
        for inst in insts:
            si = inst.sync_info
            if si is not None and len(si.on_wait) > 1:
                waits = list(si.on_wait)
                for w in waits[:-1]:
                    _WS_CTR[0] += 1
                    new_list.append(mybir.InstNoOp(
                        name=f"waitsplit-{_WS_CTR[0]}",
                        engine=inst.engine,
                        bass_nofuse=True,
                        sync_info=mybir.SyncInfo(on_wait=[w], on_update=[])))
                inst.sync_info = mybir.SyncInfo(
                    on_wait=[waits[-1]], on_update=list(si.on_update))
            new_list.append(inst)
        blk.instructions = new_list


# ---------------------------------------------------------------------------
# host side
# ---------------------------------------------------------------------------

def _rope_tables(positions):
    half = RD // 2
    inv_freq = 1.0 / (THETA ** (np.arange(half, dtype=np.float64) / half))
    ang = positions[None, :].astype(np.float64) * inv_freq[:, None]  # [32, L]
    cos_t = np.repeat(np.cos(ang), 2, axis=0).astype(np.float32)
    sin_t = np.repeat(np.sin(ang), 2, axis=0).astype(np.float32)
    sin_t[0::2] *= -1.0                                  # a-rows get -sin
    return cos_t, sin_t


def _prep_inputs(hidden, wq, wkv, wgate, ape, sinks, wo):
    bf = ml_dtypes.bfloat16
    cosq_t, sinq_t = _rope_tables(np.arange(S))
    coskf, sinkf = _rope_tables(np.arange(NW) * RATIO)   # full tables
    pw, ft = np.meshgrid(np.arange(WCH), np.arange(TCH), indexing="ij")
    band = (ft >= RATIO * pw + RATIO - 1).astype(np.float32)     # [WCH, TCH]
    eape = np.empty((HD, 2 * RATIO), np.float32)
    for r in range(RATIO):
        eape[:, r] = np.exp(ape[r, :HD])
        eape[:, RATIO + r] = np.exp(ape[r, HD:])
    maps = []
    for c in range(N_CORES):
        b, g = divmod(c, HPC)
        esink = np.exp(sinks[g * HPC:(g + 1) * HPC]).astype(np.float32)
        hT = hidden[b].T                                    # [HID, S]
        own = (g, 4 + g)
        htkv = np.zeros((HID, 2, RATIO + TCH), np.float32)
        cosk_t = np.zeros((RD, NW), np.float32)
        sink_t = np.zeros((RD, NW), np.float32)
        for oc, j in enumerate(own):
            lo = j * TCH - RATIO
            if lo < 0:
                htkv[:, oc, RATIO:] = hT[:, 0:j * TCH + TCH]
            else:
                htkv[:, oc, :] = hT[:, lo:j * TCH + TCH]
            wsl = slice(j * WCH, (j + 1) * WCH)
            cosk_t[:, oc * WCH:(oc + 1) * WCH] = coskf[:, wsl]
            sink_t[:, oc * WCH:(oc + 1) * WCH] = sinkf[:, wsl]
        gfix = np.full((128, RATIO), -30000.0 if g == 0 else 0.0, np.float32)
        maps.append({
            "ht": np.ascontiguousarray(hT).astype(bf),
            "htkv": htkv.astype(bf),
            "gfix": gfix,
            "wq": np.ascontiguousarray(wq[:, g * CW:(g + 1) * CW]).astype(bf),
            "wkv": wkv.astype(bf),
            "wg": wgate.astype(bf),
            "wo": np.ascontiguousarray(wo[g * CW:(g + 1) * CW, :]).astype(bf),
            "eape": eape,
            "esinkb": np.broadcast_to(esink[None, :], (128, HPC)).copy(),
            "cosq": cosq_t.astype(bf), "sinq": sinq_t.astype(bf),
            "cosk": cosk_t.astype(bf), "sink": sink_t.astype(bf),
            "bandm": band.astype(bf),
        })
    return maps


_RUNNER_CACHE = {}


def _get_runner(n_reps: int = 1):
    if n_reps in _RUNNER_CACHE:
        return _RUNNER_CACHE[n_reps]
    import jax
    from jax.sharding import Mesh, PartitionSpec
    from jax.experimental.shard_map import shard_map
    from concourse.bass2jax import (_bass_exec_p, install_neuronx_cc_hook,
                                    partition_id_tensor)

    nc = _build_nc(n_reps)
    install_neuronx_cc_hook()
    partition_name = nc.partition_id_tensor.name if nc.partition_id_tensor else None
    in_names, out_names, out_avals, zero_outs = [], [], [], []
    for alloc in nc.m.functions[0].allocations:
        if not isinstance(alloc, mybir.MemoryLocationSet):
            continue
        name = alloc.memorylocations[0].name
        if alloc.kind == "ExternalInput":
            if name != partition_name:
                in_names.append(name)
        elif alloc.kind == "ExternalOutput":
            out_names.append(name)
            shape = tuple(alloc.tensor_shape)
            dtype = mybir.dt.np(alloc.dtype)
            out_avals.append(jax.core.ShapedArray(shape, dtype))
            zero_outs.append(np.zeros(shape, dtype))
    n_params = len(in_names)
    all_in_names = list(in_names) + out_names
    if partition_name is not None:
        all_in_names.append(partition_name)

    def _kernel_body(*args):
        operands = list(args)
        if partition_name is not None:
            operands.append(partition_id_tensor())
        outs = _bass_exec_p.bind(
            *operands,
            out_avals=tuple(out_avals),
            in_names=tuple(all_in_names),
            out_names=tuple(out_names),
            lowering_input_output_aliases=(),
            sim_require_finite=True,
            sim_require_nnan=True,
            nc=nc,
        )
        return tuple(outs)

    devices = jax.devices()[:N_CORES]
    mesh = Mesh(np.asarray(devices), ("core",))
    spec = PartitionSpec("core")
    fn = jax.jit(shard_map(
        _kernel_body, mesh=mesh,
        in_specs=(spec,) * (n_params + len(out_names)),
        out_specs=(spec,) * len(out_names), check_rep=False))
    runner = (fn, in_names, out_names, zero_outs, mesh)
    _RUNNER_CACHE[n_reps] = runner
    return runner


def _run_core_maps(maps, n_reps: int = 1):
    import jax
    from jax.sharding import NamedSharding, PartitionSpec
    fn, in_names, out_names, zero_outs, mesh = _get_runner(n_reps)
    sh = NamedSharding(mesh, PartitionSpec("core"))
    args = [jax.device_put(
        np.concatenate([np.asarray(m[name]) for m in maps], axis=0), sh)
        for name in in_names]
    for z in zero_outs:
        args.append(jax.device_put(
            np.zeros((N_CORES * z.shape[0], *z.shape[1:]), z.dtype), sh))
    res = fn(*args)
    jax.block_until_ready(res)
    return np.asarray(res[0]).reshape(N_CORES, S, HID)


def kernel(hidden, wq, wkv, wgate, ape, sinks, wo,
           ratio=RATIO, head_dim=HD, rope_head_dim=RD, num_heads=NH):
    hidden = np.asarray(hidden, np.float32)
    maps = _prep_inputs(hidden, np.asarray(wq, np.float32),
                        np.asarray(wkv, np.float32),
                        np.asarray(wgate, np.float32),
                        np.asarray(ape, np.float32),
                        np.asarray(sinks, np.float32),
                        np.asarray(wo, np.float32))
    partials = _run_core_maps(maps)
    out = np.empty((B, S, HID), np.float32)
    for b in range(B):
        out[b] = partials[b * HPC:(b + 1) * HPC].astype(np.float64).sum(
            axis=0).astype(np.float32)
    return out


# revision 14
# speedup vs baseline: 1.1573x; 1.1573x over previous
"""DeepseekV4-style sparse attention on 8 Trainium2 cores (Bass/Tile), v2.

Sharding: data-parallel over batch (2) x tensor-parallel over heads (16 -> 4
groups of 4).  Core c handles batch c//4 and heads [4*(c%4), 4*(c%4)+4).
Per-core partial outputs (attn_heads @ wo_rows) are summed on the host.

v2 structure (vs v1): fully software-pipelined per 512-token chunk so the PE
engine never idles behind the pooling chain:
    for j: emit B_{j-1} (pooling, ACT/DVE) ; A_j (projections, PE) ;
           C_{j-1} (attention + out-proj, PE)
  - softmax denominator is computed as ones[128,128]^T @ exp(S^T), giving the
    denominator replicated across all 128 partitions ("rbden") -- no separate
    broadcast matmul / copy; sink is added via DVE tensor_scalar, then
    reciprocal, then one DVE multiply normalizes PV.
  - kv/gate slabs for pooling live in 2-chunk rings with a RATIO-column halo
    (lo halves only -- the hi halves pool the current window).
  - initial DMAs are split per-k-group and ordered so the first matmul can
    start after ~1/4 of wq + hslab0 arrived.
"""

import numpy as np
import ml_dtypes

import concourse.bass as bass
import concourse.mybir as mybir
import concourse.tile as tile
from concourse.bass import ts
from concourse.masks import make_identity

F32 = mybir.dt.float32
BF16 = mybir.dt.bfloat16
AF = mybir.ActivationFunctionType

B, S, HID, NH, HD, RD, RATIO = 2, 4096, 2048, 16, 128, 64, 4
THETA = 10000.0
NW = S // RATIO
N_CORES = 8
HPC = 4
CW = HPC * HD
TCH = 512
NCH = S // TCH
WCH = 128
KCH = HID // 128
SCALE = HD ** -0.5

_PAIR_SWAP = [i ^ 1 for i in range(32)]


def _build_nc(n_reps: int = 1, split_waits: bool = True):
    nc = bass.Bass(num_devices=N_CORES)
    dp = nc.declare_dram_parameter
    ht = dp("ht", [HID, S], BF16, isOutput=False)
    htkv = dp("htkv", [HID, 2, RATIO + TCH], BF16, isOutput=False)
    gfix = dp("gfix", [128, RATIO], F32, isOutput=False)
    wq = dp("wq", [HID, CW], BF16, isOutput=False)
    wkv = dp("wkv", [HID, 2 * HD], BF16, isOutput=False)
    wg = dp("wg", [HID, 2 * HD], BF16, isOutput=False)
    wo = dp("wo", [CW, HID], BF16, isOutput=False)
    eape = dp("eape", [HD, 2 * RATIO], F32, isOutput=False)
    esinkb = dp("esinkb", [128, HPC], F32, isOutput=False)
    cosq = dp("cosq", [RD, S], BF16, isOutput=False)
    sinq = dp("sinq", [RD, S], BF16, isOutput=False)
    cosk = dp("cosk", [RD, NW], BF16, isOutput=False)
    sink = dp("sink", [RD, NW], BF16, isOutput=False)
    bandm = dp("bandm", [WCH, TCH], BF16, isOutput=False)
    out = dp("out", [S, HID], BF16, isOutput=True)
    args = (ht, htkv, wq, wkv, wg, wo, eape, esinkb, gfix, cosq, sinq, cosk,
            sink, bandm, out)

    with tile.TileContext(nc) as tc:
        for _ in range(n_reps):   # python-unrolled: For_i + collectives
            _body(nc, tc, *args)  # fails walrus codegen
    if split_waits:
        _split_multi_waits(nc)
    return nc


def _body(nc, tc, ht, htkv, wq, wkv, wg, wo, eape, esinkb, gfix,
          cosq, sinq, cosk, sink, bandm, out):
    HLO = RATIO + TCH               # own-chunk slab width (4-col halo)
    GRP = [[0, 1, 2, 3], [4, 5, 6, 7]]
    with (
        tc.tile_pool(name="persist", bufs=1) as pp,
        tc.tile_pool(name="wts", bufs=1) as wts,
        tc.tile_pool(name="hslab", bufs=2) as hs,
        tc.tile_pool(name="ev", bufs=3) as ev,
        tc.tile_pool(name="bwork", bufs=1) as bw,
        tc.tile_pool(name="att", bufs=2) as att,
        tc.tile_pool(name="dram", bufs=2, space="DRAM") as dram,
        tc.tile_pool(name="psAO", bufs=3, space="PSUM") as psAO,
        tc.tile_pool(name="psS", bufs=2, space="PSUM") as psS,
        tc.tile_pool(name="psAcc", bufs=2, space="PSUM") as psAcc,
        tc.tile_pool(name="psRB", bufs=1, space="PSUM") as psRB,
    ):
        # ---- persistent SBUF state ----
        qT = [pp.tile([128, 4, TCH], BF16, tag=f"qT{m}", name=f"qT{m}")
              for m in range(HPC)]
        # own-chunk kv/gate slabs (2 owned chunks, halo cols 0:RATIO)
        okvlo = pp.tile([128, 2, HLO], BF16, tag="okvlo", name="okvlo")
        oglo = pp.tile([128, 2, HLO], BF16, tag="oglo", name="oglo")
        okvhi = pp.tile([128, 2, HLO], BF16, tag="okvhi", name="okvhi")
        oghi = pp.tile([128, 2, HLO], BF16, tag="oghi", name="oghi")
        cosq_s = pp.tile([128, S], BF16, tag="cosq", name="cosq")
        sinq_s = pp.tile([128, S], BF16, tag="sinq", name="sinq")
        cosk_s = pp.tile([128, 2 * WCH], BF16, tag="cosk", name="cosk")
        sink_s = pp.tile([128, 2 * WCH], BF16, tag="sink", name="sink")
        eape_s = pp.tile([HD, 2 * RATIO], F32, tag="eape", name="eape")
        esink_s = pp.tile([128, HPC], F32, tag="esink", name="esink")
        gfix_s = pp.tile([128, RATIO], F32, tag="gfix", name="gfix")
        bandm_s = pp.tile([WCH, TCH], BF16, tag="bandm", name="bandm")
        wo_s = pp.tile([HD, HPC, HID], BF16, tag="wo", name="wo")
        ones_w = pp.tile([WCH, WCH], BF16, tag="ones_w", name="ones_w")
        kT = pp.tile([HD, NW], BF16, tag="kT", name="kT")
        v_s = pp.tile([WCH, NW // WCH, HD], BF16, tag="v", name="v")
        ident = pp.tile([128, 128], F32, tag="ident", name="ident")

        wq_s = wts.tile([128, KCH, CW], BF16, tag="wq", name="wq")
        wkv_s = wts.tile([128, KCH, 2 * HD], BF16, tag="wkv", name="wkv")
        wg_s = wts.tile([128, KCH, 2 * HD], BF16, tag="wg", name="wg")
        htkv_s = [wts.tile([128, KCH, HLO], BF16, tag=f"htkv{oc}",
                           name=f"htkv{oc}") for oc in range(2)]

        # ---- preamble: ordered DMAs (chunk-0-critical first) ----
        htr = ht.rearrange("(k p) t -> p k t", p=128)
        htkvr = htkv.rearrange("(k p) o w -> p k o w", p=128)
        wqr = wq.rearrange("(k p) c -> p k c", p=128)
        hsl0 = hs.tile([128, KCH, TCH], BF16, tag="hslab", name="hslab0")
        for g in range(4):
            ksl = slice(4 * g, 4 * g + 4)
            nc.sync.dma_start(wq_s[:, ksl, :], wqr[:, ksl, :])
            nc.sync.dma_start(hsl0[:, ksl, :], htr[:, ksl, ts(0, TCH)])
        nc.sync.dma_start(wkv_s[:], wkv.rearrange("(k p) c -> p k c", p=128))
        nc.sync.dma_start(wg_s[:], wg.rearrange("(k p) c -> p k c", p=128))
        for g in range(4):
            ksl = slice(4 * g, 4 * g + 4)
            nc.sync.dma_start(htkv_s[0][:, ksl, :], htkvr[:, ksl, 0, :])
        nc.sync.dma_start(cosq_s[RD:128, ts(0, TCH)], cosq[:, ts(0, TCH)])
        nc.sync.dma_start(sinq_s[RD:128, ts(0, TCH)], sinq[:, ts(0, TCH)])
        nc.sync.dma_start(gfix_s[:], gfix[:])
        nc.sync.dma_start(eape_s[:], eape[:])
        nc.sync.dma_start(esink_s[:], esinkb[:])
        nc.sync.dma_start(cosk_s[RD:128, :], cosk[:, 0:2 * WCH])
        nc.sync.dma_start(sink_s[RD:128, :], sink[:, 0:2 * WCH])
        nc.vector.memset(ones_w[:], 1.0)
        make_identity(nc, ident[:])

        hsl_tiles = {0: hsl0}
        pk_outs = {}

        def stage_aq(j):
            """q projection (+RoPE) for chunk j; prefetch next hslab."""
            tsl = ts(j, TCH)
            hsl = hsl_tiles.pop(j)
            if j + 1 < NCH:
                nxt = hs.tile([128, KCH, TCH], BF16, tag="hslab",
                              name=f"hslab{j + 1}")
                for g in range(4):
                    ksl = slice(4 * g, 4 * g + 4)
                    nc.sync.dma_start(nxt[:, ksl, :],
                                      htr[:, ksl, ts(j + 1, TCH)])
                hsl_tiles[j + 1] = nxt
                nc.sync.dma_start(cosq_s[RD:128, ts(j + 1, TCH)],
                                  cosq[:, ts(j + 1, TCH)])
                nc.sync.dma_start(sinq_s[RD:128, ts(j + 1, TCH)],
                                  sinq[:, ts(j + 1, TCH)])
            if j == 0:      # heavy non-critical loads after hslab1 prefetch
                for g in range(4):
                    ksl = slice(4 * g, 4 * g + 4)
                    nc.sync.dma_start(htkv_s[1][:, ksl, :],
                                      htkvr[:, ksl, 1, :])
                nc.sync.dma_start(bandm_s[:], bandm[:])
                nc.sync.dma_start(wo_s[:],
                                  wo.rearrange("(h p) e -> p h e", p=HD))
            sl = j % 4
            for m in range(HPC):
                ps = psAO.tile([128, TCH], F32, tag="a", name="aq")
                for k in range(KCH):
                    nc.tensor.matmul(ps[:], wq_s[:, k, ts(m, 128)],
                                     hsl[:, k, :], start=(k == 0),
                                     stop=(k == KCH - 1))
                nc.scalar.copy(qT[m][0:RD, sl, :], ps[0:RD, :])
                rb = ev.tile([128, TCH], BF16, tag="ropebuf", name="ropebuf", bufs=2)
                nc.vector.tensor_copy(rb[RD:128, :], ps[RD:128, :])
                sw = ev.tile([128, TCH], BF16, tag="ropeswap", name="ropeswap", bufs=2)
                nc.vector.stream_shuffle(sw[RD:128, :], rb[RD:128, :],
                                         _PAIR_SWAP)
                t1 = ev.tile([128, TCH], BF16, tag="ropet1", name="ropet1", bufs=2)
                nc.vector.tensor_mul(t1[RD:128, :], rb[RD:128, :],
                                     cosq_s[RD:128, tsl])
                t2 = ev.tile([128, TCH], BF16, tag="ropet2", name="ropet2", bufs=2)
                nc.vector.tensor_mul(t2[RD:128, :], sw[RD:128, :],
                                     sinq_s[RD:128, tsl])
                nc.vector.tensor_add(qT[m][RD:128, sl, :], t1[RD:128, :],
                                     t2[RD:128, :])

        def stage_own(oc):
            """kv/gate projection + pooling + rope/transpose + pack for the
            core's own chunk `oc` (0 or 1); ends with the pack DMA to DRAM."""
            for dst, col, wsrc in (
                (okvlo, slice(0, 128), wkv_s),
                (okvhi, slice(128, 256), wkv_s),
                (oglo, slice(0, 128), wg_s),
                (oghi, slice(128, 256), wg_s),
            ):
                ps = psAO.tile([128, TCH], F32, tag="a", name="okv")
                for k in range(KCH):
                    nc.tensor.matmul(ps[:], wsrc[:, k, col],
                                     htkv_s[oc][:, k, 0:TCH],
                                     start=(k == 0), stop=(k == KCH - 1))
                psh = psAO.tile([128, TCH], F32, tag="a", name="okvh")
                for k in range(KCH):
                    nc.tensor.matmul(psh[:, 0:RATIO], wsrc[:, k, col],
                                     htkv_s[oc][:, k, TCH:HLO],
                                     start=(k == 0), stop=(k == KCH - 1))
                nc.scalar.copy(dst[:, oc, 0:TCH], ps[:])
                nc.scalar.copy(dst[:, oc, TCH:HLO], psh[:, 0:RATIO])
            if oc == 0:   # chunk-0 halo gate -> -inf on the owning cores
                nc.vector.tensor_add(oglo[:, 0, 0:RATIO],
                                     oglo[:, 0, 0:RATIO], gfix_s[:])


            # pooling (windows of the owned chunk)
            numer = bw.tile([HD, WCH], F32, tag="numer", name="numer")
            denom = bw.tile([HD, WCH], F32, tag="denom", name="denom")
            for half, (g_src, kv_src, acol, off) in enumerate((
                (oglo[:, oc, :], okvlo[:, oc, :], slice(0, RATIO), 0),
                (oghi[:, oc, :], okvhi[:, oc, :], slice(RATIO, 2 * RATIO),
                 RATIO),
            )):
                e = bw.tile([HD, HLO], F32, tag="poole", name=f"poole{half}")
                nc.scalar.activation(e[:], g_src[:], AF.Exp)
                nc.vector.tensor_mul(
                    e[:].rearrange("d (w r) -> d w r", r=RATIO),
                    e[:].rearrange("d (w r) -> d w r", r=RATIO),
                    eape_s[:, None, acol].to_broadcast(
                        [HD, HLO // RATIO, RATIO]))
                ea = bw.tile([HD, HLO], F32, tag="poolea",
                             name=f"poolea{half}")
                nc.vector.tensor_mul(ea[:], e[:], kv_src[:])
                for acc, src in ((denom, e), (numer, ea)):
                    s3 = src[:, off:off + TCH].rearrange(
                        "d (w r) -> d w r", r=RATIO)
                    nm = f"poolred{half}{1 if acc is numer else 0}"
                    ra = bw.tile([HD, WCH], F32, tag="poolra", name=nm + "a")
                    nc.vector.tensor_add(ra[:], s3[:, :, 0], s3[:, :, 1])
                    rc = bw.tile([HD, WCH], F32, tag="poolrc", name=nm + "c")
                    nc.vector.tensor_add(rc[:], s3[:, :, 2], s3[:, :, 3])
                    if half == 0:
                        nc.vector.tensor_add(acc[:], ra[:], rc[:])
                    else:
                        nc.vector.tensor_add(ra[:], ra[:], rc[:])
                        nc.vector.tensor_add(acc[:], acc[:], ra[:])
            rec = bw.tile([HD, WCH], F32, tag="poolrec", name="poolrec")
            nc.vector.reciprocal(rec[:], denom[:])
            pooledc = bw.tile([HD, WCH], F32, tag="pooledc", name="pooledc")
            nc.vector.tensor_mul(pooledc[:], numer[:], rec[:])

            pack = bw.tile([128, 2 * WCH], BF16, tag="pack", name="pack",
                           bufs=2)
            # cols 0:WCH = roped kT chunk; WCH:2*WCH = V chunk (transposed).
            # rope needs per-chunk position tables: owned chunk index is
            # data-dependent, so host sends cosk/sink laid out per OWN chunk.
            nc.scalar.copy(pack[0:RD, 0:WCH], pooledc[0:RD, :])
            krb = bw.tile([128, WCH], BF16, tag="krope", name="krope")
            nc.scalar.copy(krb[RD:128, :], pooledc[RD:128, :])
            ksw = bw.tile([128, WCH], BF16, tag="kswap", name="kswap")
            nc.vector.stream_shuffle(ksw[RD:128, :], krb[RD:128, :],
                                     _PAIR_SWAP)
            kt1 = bw.tile([128, WCH], BF16, tag="kt1", name="kt1")
            nc.vector.tensor_mul(kt1[RD:128, :], krb[RD:128, :],
                                 cosk_s[RD:128, ts(oc, WCH)])
            kt2 = bw.tile([128, WCH], BF16, tag="kt2", name="kt2")
            nc.vector.tensor_mul(kt2[RD:128, :], ksw[RD:128, :],
                                 sink_s[RD:128, ts(oc, WCH)])
            nc.vector.tensor_add(pack[RD:128, 0:WCH], kt1[RD:128, :],
                                 kt2[RD:128, :])
            pk_in = dram.tile([128, 2 * WCH], BF16, tag="pk_in",
                              name=f"pk_in{oc}")
            nc.gpsimd.dma_start(pk_in[:, 0:WCH], pack[:, 0:WCH])
            return pooledc, pack, pk_in

        def stage_own_fin(oc, pooledc, pack, pk_in):
            # deferred PE transpose (runs after the next q-projection)
            tp = psAO.tile([128, TCH], F32, tag="a", name="vtrans")
            nc.tensor.transpose(tp[:, 0:WCH], pooledc[:], ident[:])
            nc.scalar.copy(pack[:, WCH:2 * WCH], tp[:, 0:WCH])
            nc.gpsimd.dma_start(pk_in[:, WCH:2 * WCH], pack[:, WCH:2 * WCH])
            return pk_in

        def gather(oc, pk_in):
            pk_out = dram.tile([4 * 128, 2 * WCH], BF16, tag="pk_out",
                               name=f"pk_out{oc}")
            nc.gpsimd.collective_compute(
                "AllGather", mybir.AluOpType.bypass, replica_groups=GRP,
                ins=[pk_in[:].opt()], outs=[pk_out[:].opt()])
            pk_outs[oc] = pk_out

        def unpack(oc):
            pk_out = pk_outs.pop(oc)
            for r in range(4):
                ch = 4 * oc + r
                nc.sync.dma_start(kT[:, ts(ch, WCH)],
                                  pk_out[128 * r:128 * (r + 1), 0:WCH])
                nc.sync.dma_start(v_s[:, ch, :],
                                  pk_out[128 * r:128 * (r + 1), WCH:2 * WCH])

        def stage_c_head(jc, h, a_sb):
            """Score/PV/denominator loop + normalize for one head of chunk jc."""
            qsl = qT[h][:, jc % 4, :]
            if True:
                rbden = psRB.tile([128, TCH], F32, tag="rb", name="rbden")
                acc_ps = psAcc.tile([128, TCH], F32, tag="acc", name="acc")
                s_tiles = {}

                def _score(wc, s_tiles=s_tiles, qsl=qsl):
                    sp = psS.tile([WCH, TCH], F32, tag="s", name="s")
                    nc.tensor.matmul(sp[:], kT[:, ts(wc, WCH)], qsl,
                                     start=True, stop=True)
                    s_tiles[wc] = sp

                _score(0)
                for wc in range(jc + 1):
                    if wc < jc:
                        _score(wc + 1)
                    s_ps = s_tiles.pop(wc)
                    pt = ev.tile([WCH, TCH], BF16, tag="pt", name="pt")
                    nc.scalar.activation(pt[:], s_ps[:], AF.Exp, scale=SCALE)
                    if wc == jc:
                        ptm = ev.tile([WCH, TCH], BF16, tag="ptm", name="ptm")
                        nc.vector.tensor_mul(ptm[:], pt[:], bandm_s[:])
                        pt = ptm
                    nc.tensor.matmul(rbden[:], ones_w[:], pt[:],
                                     start=(wc == 0), stop=(wc == jc))
                    nc.tensor.matmul(acc_ps[:], v_s[:, wc, :], pt[:],
                                     start=(wc == 0), stop=(wc == jc))
                dsink = att.tile([128, TCH], F32, tag="dsink", name="dsink")
                nc.scalar.activation(dsink[:], rbden[:], AF.Identity,
                                     bias=esink_s[:, h:h + 1])
                rec_sb = att.tile([128, TCH], F32, tag="rec_sb", name="rec_sb")
                nc.vector.reciprocal(rec_sb[:], dsink[:])
                a = att.tile([128, TCH], BF16, tag=f"attnT{h}",
                             name=f"attnT{h}")
                nc.vector.tensor_mul(a[:], acc_ps[:], rec_sb[:])
                a_sb.append(a)

        def stage_c_o(jc, a_sb, tts):
            for tt in tts:
                for e in range(HID // TCH):
                    o_ps = psAO.tile([128, TCH], F32, tag="a", name="o")
                    for h in range(HPC):
                        nc.tensor.matmul(o_ps[:], a_sb[h][:, ts(tt, 128)],
                                         wo_s[:, h, ts(e, TCH)],
                                         start=(h == 0), stop=(h == HPC - 1))
                    o_sb = att.tile([128, TCH], BF16, tag="o_sb", name="o_sb",
                                    bufs=2)
                    if e % 2 == 0:
                        nc.scalar.copy(o_sb[:], o_ps[:])
                    else:
                        nc.vector.tensor_copy(o_sb[:], o_ps[:])
                    nc.sync.dma_start(
                        out[jc * TCH + tt * 128:jc * TCH + (tt + 1) * 128,
                            ts(e, TCH)], o_sb[:])

        def stage_c(jc):
            a_sb = []
            for h in range(HPC):
                stage_c_head(jc, h, a_sb)
            stage_c_o(jc, a_sb, range(TCH // 128))
            return a_sb

        pk0 = pk1 = None
        for j in range(NCH):
            stage_aq(j)
            if j == 0:
                own0 = stage_own(0)
            elif j == 1:
                pk0 = stage_own_fin(0, *own0)
                own1 = stage_own(1)
                gather(0, pk0)
                unpack(0)
            elif j == 2:
                pk1 = stage_own_fin(1, *own1)
                gather(1, pk1)
                unpack(1)
            if j >= 3:
                stage_c(j - 3)
        a_prev = None
        for jc in range(NCH - 3, NCH):
            a_cur = []
            for h in range(HPC):
                stage_c_head(jc, h, a_cur)
                if a_prev is not None:
                    stage_c_o(jc - 1, a_prev, [h])
            a_prev = a_cur
        stage_c_o(NCH - 1, a_prev, range(TCH // 128))
_WS_CTR = [0]


def _split_multi_waits(nc):
    """This walrus build accepts at most ONE sync wait per instruction; hoist
    extras onto same-engine NOPs placed immediately before."""
    f = nc.m.functions[0]
    for blk in f.blocks:
        insts = blk.instructions
        if not any(i.sync_info is not None and len(i.sync_info.on_wait) > 1
                   for i in insts):
            continue
        new_list = []
        for inst in insts:
            si = inst.sync_info
            if si is not None and len(si.on_wait) > 1:
                waits = list(si.on_wait)
                for w in waits[:-1]:
                    _WS_CTR[0] += 1
                    new_list.append(mybir.InstNoOp(
                        name=f"waitsplit-{_WS_CTR[0]}",
                        engine=inst.engine,
                        bass_nofuse=True,
                        sync_info=mybir.SyncInfo(on_wait=[w], on_update=[])))
                inst.sync_info = mybir.SyncInfo(
                    on_wait=[waits[-1]], on_update=list(si.on_update))
            new_list.append(inst)
        blk.instructions = new_list


# ---------------------------------------------------------------------------
# host side
# ---------------------------------------------------------------------------

def _rope_tables(positions):
    half = RD // 2
    inv_freq = 1.0 / (THETA ** (np.arange(half, dtype=np.float64) / half))
    ang = positions[None, :].astype(np.float64) * inv_freq[:, None]  # [32, L]
    cos_t = np.repeat(np.cos(ang), 2, axis=0).astype(np.float32)
    sin_t = np.repeat(np.sin(ang), 2, axis=0).astype(np.float32)
    sin_t[0::2] *= -1.0                                  # a-rows get -sin
    return cos_t, sin_t


def _prep_inputs(hidden, wq, wkv, wgate, ape, sinks, wo):
    bf = ml_dtypes.bfloat16
    cosq_t, sinq_t = _rope_tables(np.arange(S))
    coskf, sinkf = _rope_tables(np.arange(NW) * RATIO)   # full tables
    pw, ft = np.meshgrid(np.arange(WCH), np.arange(TCH), indexing="ij")
    band = (ft >= RATIO * pw + RATIO - 1).astype(np.float32)     # [WCH, TCH]
    eape = np.empty((HD, 2 * RATIO), np.float32)
    for r in range(RATIO):
        eape[:, r] = np.exp(ape[r, :HD])
        eape[:, RATIO + r] = np.exp(ape[r, HD:])
    maps = []
    for c in range(N_CORES):
        b, g = divmod(c, HPC)
        esink = np.exp(sinks[g * HPC:(g + 1) * HPC]).astype(np.float32)
        hT = hidden[b].T                                    # [HID, S]
        own = (g, 4 + g)
        htkv = np.zeros((HID, 2, RATIO + TCH), np.float32)
        cosk_t = np.zeros((RD, NW), np.float32)
        sink_t = np.zeros((RD, NW), np.float32)
        for oc, j in enumerate(own):
            lo = j * TCH - RATIO
            if lo < 0:
                htkv[:, oc, RATIO:] = hT[:, 0:j * TCH + TCH]
            else:
                htkv[:, oc, :] = hT[:, lo:j * TCH + TCH]
            wsl = slice(j * WCH, (j + 1) * WCH)
            cosk_t[:, oc * WCH:(oc + 1) * WCH] = coskf[:, wsl]
            sink_t[:, oc * WCH:(oc + 1) * WCH] = sinkf[:, wsl]
        gfix = np.full((128, RATIO), -30000.0 if g == 0 else 0.0, np.float32)
        maps.append({
            "ht": np.ascontiguousarray(hT).astype(bf),
            "htkv": htkv.astype(bf),
            "gfix": gfix,
            "wq": np.ascontiguousarray(wq[:, g * CW:(g + 1) * CW]).astype(bf),
            "wkv": wkv.astype(bf),
            "wg": wgate.astype(bf),
            "wo": np.ascontiguousarray(wo[g * CW:(g + 1) * CW, :]).astype(bf),
            "eape": eape,
            "esinkb": np.broadcast_to(esink[None, :], (128, HPC)).copy(),
            "cosq": cosq_t.astype(bf), "sinq": sinq_t.astype(bf),
            "cosk": cosk_t.astype(bf), "sink": sink_t.astype(bf),
            "bandm": band.astype(bf),
        })
    return maps


_RUNNER_CACHE = {}


def _get_runner(n_reps: int = 1):
    if n_reps in _RUNNER_CACHE:
        return _RUNNER_CACHE[n_reps]
    import jax
    from jax.sharding import Mesh, PartitionSpec
    from jax.experimental.shard_map import shard_map
    from concourse.bass2jax import (_bass_exec_p, install_neuronx_cc_hook,
                                    partition_id_tensor)

    nc = _build_nc(n_reps)
    install_neuronx_cc_hook()
    partition_name = nc.partition_id_tensor.name if nc.partition_id_tensor else None
    in_names, out_names, out_avals, zero_outs = [], [], [], []
    for alloc in nc.m.functions[0].allocations:
        if not isinstance(alloc, mybir.MemoryLocationSet):
            continue
        name = alloc.memorylocations[0].name
        if alloc.kind == "ExternalInput":
            if name != partition_name:
                in_names.append(name)
        elif alloc.kind == "ExternalOutput":
            out_names.append(name)
            shape = tuple(alloc.tensor_shape)
            dtype = mybir.dt.np(alloc.dtype)
            out_avals.append(jax.core.ShapedArray(shape, dtype))
            zero_outs.append(np.zeros(shape, dtype))
    n_params = len(in_names)
    all_in_names = list(in_names) + out_names
    if partition_name is not None:
        all_in_names.append(partition_name)

    def _kernel_body(*args):
        operands = list(args)
        if partition_name is not None:
            operands.append(partition_id_tensor())
        outs = _bass_exec_p.bind(
            *operands,
            out_avals=tuple(out_avals),
            in_names=tuple(all_in_names),
            out_names=tuple(out_names),
            lowering_input_output_aliases=(),
            sim_require_finite=True,
            sim_require_nnan=True,
            nc=nc,
        )
        return tuple(outs)

    devices = jax.devices()[:N_CORES]
    mesh = Mesh(np.asarray(devices), ("core",))
    spec = PartitionSpec("core")
    fn = jax.jit(shard_map(
        _kernel_body, mesh=mesh,
        in_specs=(spec,) * (n_params + len(out_names)),
        out_specs=(spec,) * len(out_names), check_rep=False))
    runner = (fn, in_names, out_names, zero_outs, mesh)
    _RUNNER_CACHE[n_reps] = runner
    return runner


def _run_core_maps(maps, n_reps: int = 1):
    import jax
    from jax.sharding import NamedSharding, PartitionSpec
    fn, in_names, out_names, zero_outs, mesh = _get_runner(n_reps)
    sh = NamedSharding(mesh, PartitionSpec("core"))
    args = [jax.device_put(
        np.concatenate([np.asarray(m[name]) for m in maps], axis=0), sh)
        for name in in_names]
    for z in zero_outs:
        args.append(jax.device_put(
            np.zeros((N_CORES * z.shape[0], *z.shape[1:]), z.dtype), sh))
    res = fn(*args)
    jax.block_until_ready(res)
    return np.asarray(res[0]).reshape(N_CORES, S, HID)


def kernel(hidden, wq, wkv, wgate, ape, sinks, wo,
           ratio=RATIO, head_dim=HD, rope_head_dim=RD, num_heads=NH):
    hidden = np.asarray(hidden, np.float32)
    maps = _prep_inputs(hidden, np.asarray(wq, np.float32),
                        np.asarray(wkv, np.float32),
                        np.asarray(wgate, np.float32),
                        np.asarray(ape, np.float32),
                        np.asarray(sinks, np.float32),
                        np.asarray(wo, np.float32))
    partials = _run_core_maps(maps)
    out = np.empty((B, S, HID), np.float32)
    for b in range(B):
        out[b] = partials[b * HPC:(b + 1) * HPC].astype(np.float64).sum(
            axis=0).astype(np.float32)
    return out
